# revision 33
# baseline (speedup 1.0000x reference)
"""Trainium2 Bass kernel: multi-head attention with quantum (cumprod-of-cos)
transform.

Full-input contract: kernel(**inputs) takes the unsharded inputs and returns
the full [B, S, E] output. Internally shards over 8 NeuronCores: data-parallel
over batch (B=2) x tensor-parallel over head-groups (4 heads per core).

v2 design (fp8 DoubleRow + engine rebalance). Per core:

Phase A, per 128-row s-tile t (16 tiles):
  - theta = x @ [Wq|Wk|Wv]^T via fp8e4 DoubleRow matmuls (x, 64*W in fp8;
    DR packs 2 contraction k-tiles per instruction at 0.5 cycles/row -> 4x
    fewer PE cycles than fp16). th psum tile [128, 2, 512] (bank0 = q|k,
    bank1 = v).
  - cos via Sin activation (bias pi/2, scale 1/64 compensating the weight
    prescale). c layout = 12 segments x 64 (q0..3, k0..3, v0..3) where each
    segment's col 0 stays ZERO in c; the boundary cos value goes to dz
    (second activation, 12 cols). This enables ONE segmented cumprod scan:
      state = c*state + dz  -> at seg start: 0*state + cos0 (reset);
    one DVE tensor_tensor_scan [128, 768] per tile instead of 12.
  - q/k scan results (fp16) are transposed by the DMA engines
    (dma_start_transpose, idle XBAR hardware) into per-sb buffers
    zqkT[sb][128, 4chunk, 4t', 128] -- no PE transposes, no psum->sbuf
    copies. v results convert fp16->fp8 on Pool (tensor_copy) into the
    vz pair tiles [128, 2t, 4h, (ones|z), 64]; ones slots are memset once.

Phase B attention (two interleaved (m, h2) streams, s-chunk-major), per
t-PAIR (8 steps per stream):
  - 2 score matmuls (fp16, K=64, tile_position row-halves) into a 2-bank
    sc2 psum tile [128, 2, 512]
  - ONE exp over the pair [128, 1024] -> ex fp8e4 [128, 2, 512]:
    alternating ACT (activation Exp, scale=1/8) and DVE (int8 Schraudolph:
    i8 = A*s + B bitcast fp8e4) to balance the two engines
  - ONE fp8 DoubleRow acc matmul per pair: lhsT = vz[ones|z] (denominator
    free via ones), rhs = ex pair; 8 DR matmuls accumulate all 16 t-tiles
  - rec merged over both streams (reciprocal_approx_fast [64, 2, 512]);
    oz = acc[64:128]*rec -> fp8 into ozb [128, 2m, S]
  - final projection: ONE fp8 DR matmul per 128-row out chunk (Wc pair
    packed), psum pairs copied fp16 (ACT/DVE alternating) and DMA'd out.

Single PSUM pool, tags "big" ([128,2,512] = th in A, sc2/py2 in B, 2 bufs)
and "acc" (2 bufs) = exactly 8 banks; no pool-transition barrier, so phase B
score matmuls overlap phase A's tail.

Weights are prescaled by 64 on host (fp8e4m3 normal range); y is divided by
64 on host. ACT tables: Sin (A) + Exp (B), 2 loads.
"""

import os
import sys

import numpy as np

if "/opt/trn_rl_repo" not in sys.path:
    sys.path.insert(0, "/opt/trn_rl_repo")

import concourse.bass as bass  # noqa: F401
import concourse.tile as tile
from concourse import bacc
from concourse import mybir
from concourse.bass_utils import run_bass_kernel_spmd

AF = mybir.ActivationFunctionType
ALU = mybir.AluOpType
F32 = mybir.dt.float32
F16 = mybir.dt.float16
F8 = mybir.dt.float8e4
I8 = mybir.dt.int8
DR = mybir.MatmulPerfMode.DoubleRow

B, S, E, H, D = 2, 2048, 1024, 16, 64
NCORES = 8
HG = 4          # heads per core
EG = HG * D     # 256
P = 128
NT = S // P     # 16 s-tiles
KC = E // P     # 8 contraction tiles for the projections
HALF_PI = float(np.pi / 2)
INV_SQRT_D = 0.125  # 1/sqrt(64)
WSCALE = 64.0       # host-side weight prescale so fp8e4m3 stays normal

# fp8e4m3 Schraudolph: exp(s/8) ~= bitcast_e4m3(i8(EXP_A*s + EXP_B));
# max rel err ~7.3%, mean 2.6% (calibrated against the executor's
# fp32->int8 truncation).
EXP_A = float(np.log2(np.e))  # 1.4427: 8 * log2(e) * (1/8)
EXP_B = 56.13
# engine pattern for the per-pair exp: ACT-heavy to balance DVE's scan +
# rec/oz load. idx % EXP_MOD in EXP_DVE -> DVE Schraudolph, else ACT.
EXP_MOD = 8
EXP_DVE = (1, 4)

_DEBUG = bool(int(os.environ.get("QK_DEBUG", "0")))


def _phase_a(tc, x8, w8s, hp, warm, c_bufs, dz_bufs, zqs, zqkT, vz, ps):
    nc = tc.nc
    # PE p-state warm-up: ~6 us of dep-free fp32 matmuls during the DMA
    # lead-in so the theta matmuls run at the full 2.4 GHz p-state.
    wps = ps.tile([P, 2, 512], F32, tag="big", bufs=3, name="warmps")
    for wi in range(4):
        nc.tensor.matmul(wps[0:1, 0, :], lhsT=warm[:, 0:1], rhs=warm[:],
                         start=True, stop=True, skip_group_check=True)
    for t in range(NT):
        tsl = slice(t * P, (t + 1) * P)
        th = ps.tile([P, 2, 512], F32, tag="big", bufs=3, name=f"th{t}")
        for j in range(4):
            nc.tensor.matmul(
                th[:, 0, :],
                lhsT=x8[:, 2 * j:2 * j + 2, tsl],
                rhs=w8s[:, 2 * j:2 * j + 2, 0:2 * EG],
                start=(j == 0), stop=(j == 3),
                perf_mode=DR,
            )
            nc.tensor.matmul(
                th[:, 1, 0:EG],
                lhsT=x8[:, 2 * j:2 * j + 2, tsl],
                rhs=w8s[:, 2 * j:2 * j + 2, 2 * EG:3 * EG],
                start=(j == 0), stop=(j == 3),
                perf_mode=DR,
            )
        c = c_bufs[t % len(c_bufs)]
        dz = dz_bufs[t % len(dz_bufs)]
        th_seg = th[:].rearrange("p a b -> p (a b)").rearrange(
            "p (s c) -> p s c", c=D)
        c_seg = c[:].rearrange("p (s c) -> p s c", c=D)
        # main Sin: cols 1..63 of each of the 12 segments (v segs start at
        # flat col 512 in th, which is exactly seg index 8 -- uniform stride)
        nc.scalar.activation(
            c_seg[:, 0:12, 1:D], th_seg[:, 0:12, 1:D], AF.Sin,
            bias=hp[:], scale=1.0 / WSCALE)
        # boundary Sin: col 0 of each segment -> dz (c col 0 stays zero)
        dz_seg = dz[:].rearrange("p (s c) -> p s c", c=D)
        nc.scalar.activation(
            dz_seg[:, 0:12, 0:1], th_seg[:, 0:12, 0:1], AF.Sin,
            bias=hp[:], scale=1.0 / WSCALE)
        # segmented cumprod: state = c*state + dz (reset at seg starts)
        nc.vector.tensor_tensor_scan(
            zqs[:, t, :], c[:], dz[:], 0.0, ALU.mult, ALU.add)
        # v part -> fp8 into the vz pair tile (Pool)
        nc.gpsimd.tensor_copy(
            out=vz[t // 2][:, t % 2, :, 1, :],
            in_=zqs[:, t, 2 * EG:3 * EG].rearrange("p (h d) -> p h d", d=D))
        # q+k -> transposed via DMA xbar into the per-sb buffer
        nc.sync.dma_start_transpose(
            out=zqkT[t // 4][:, :, t % 4, :],
            in_=zqs[:, t, 0:2 * EG])


def _attention(tc, ozb, zqkT, vz, wc16, yT, ps, dbg=None):
    nc = tc.nc
    ex_idx = 0
    cp_idx = 0
    pending = []  # (mo, sb) final-projection chunk pairs ready to project

    with (
        tc.tile_pool(name="y", bufs=3) as yp,
        tc.tile_pool(name="norm", bufs=2) as nrm,
        tc.tile_pool(name="exps", bufs=12) as exq,
    ):
        def emit_final_pair(tag, engine=None):
            nonlocal cp_idx
            mo, sb = pending.pop(0)
            ssl = slice(sb * 512, (sb + 1) * 512)
            py2 = ps.tile([P, 2, 512], F32, tag=tag, bufs=3,
                          name=f"py{mo}_{sb}")
            for i in range(2):
                for kk in range(2):
                    nc.tensor.matmul(
                        py2[:, i, :],
                        lhsT=wc16[:, kk, (mo + i) * P:(mo + i + 1) * P],
                        rhs=ozb[:, kk, ssl],
                        start=(kk == 0), stop=(kk == 1),
                    )
            yt = yp.tile([P, 2, 512], F16, tag="y", name=f"yt{mo}_{sb}")
            if engine is None:
                engine = "a" if cp_idx % 2 == 0 else "d"
            if engine == "a":
                nc.scalar.copy(
                    out=yt[:], in_=py2[:].rearrange("p a b -> p (a b)"))
            else:
                nc.vector.tensor_copy(
                    out=yt[:], in_=py2[:].rearrange("p a b -> p (a b)"))
            cp_idx += 1
            nc.sync.dma_start(
                out=yT[mo * P:(mo + 2) * P, ssl].rearrange(
                    "(k p) s -> p k s", p=P),
                in_=yt[:])

        NP = NT // 2  # 8 t-pairs
        combos = [(m, h2, sb) for sb in range(4) for m in range(2)
                  for h2 in range(2)]
        # One stream per group: acc is a single 1-bank tile (bufs=2, so
        # group boundaries never WAR-block), and sc2's 3 buffers serve ONE
        # allocation per slot -- a 3-slot lookahead that covers the
        # score->exp->release round trip (~1.8us) at ~0.9us slots.
        LAG = 2
        recs = [None]
        deferred = []  # prev group's rec/oz closures, one per slot
        for gi, (m, h2, sb) in enumerate(combos):
            h = 2 * m + h2
            dbase = h2 * D
            ssl = slice(sb * 512, (sb + 1) * 512)
            acc = ps.tile([P, 512], F32, tag="acc", bufs=2, name=f"acc{gi}")
            exs = [None] * NP
            nfin = 0
            for tp in range(NP + LAG):
                # prev group's rec/oz, one per slot so they interleave
                # between this group's DVE exps instead of damming them
                if deferred and tp >= 1:
                    deferred.pop(0)()
                if tp < NP:
                    sc2 = ps.tile([P, 2, 512], F32, tag="big", bufs=3,
                                  name=f"sc{gi}_{tp}")
                    for tt in range(2):
                        t = 2 * tp + tt
                        nc.tensor.matmul(
                            sc2[:, tt, :],
                            lhsT=zqkT[t // 4][dbase:dbase + D, 2 + m,
                                              t % 4, :],
                            rhs=zqkT[sb][dbase:dbase + D, m, :, :]
                                .rearrange("p a b -> p (a b)"),
                            start=True, stop=True,
                            tile_position=(dbase, 0),
                        )
                    ex = exq.tile([P, 2, 512], F8, tag="ex",
                                  name=f"ex{gi}_{tp}")
                    sc_flat = sc2[:].rearrange("p a b -> p (a b)")
                    if (ex_idx % EXP_MOD) in EXP_DVE:
                        nc.vector.tensor_scalar(
                            out=ex[:].rearrange(
                                "p a b -> p (a b)").bitcast(I8),
                            in0=sc_flat,
                            scalar1=EXP_A, scalar2=EXP_B,
                            op0=ALU.mult, op1=ALU.add,
                        )
                    else:
                        nc.scalar.activation(
                            ex[:].rearrange("p a b -> p (a b)"),
                            sc_flat, AF.Exp, scale=INV_SQRT_D)
                    if _DEBUG and gi == 0 and tp == 0:
                        nc.sync.dma_start(out=dbg["dbg_ex0"][:], in_=ex[:])
                    ex_idx += 1
                    exs[tp] = ex
                if tp >= LAG:
                    tpp = tp - LAG
                    nc.tensor.matmul(
                        acc[:],
                        lhsT=vz[tpp][:, :, h, :, :].rearrange(
                            "p a b c -> p a (b c)"),
                        rhs=exs[tpp][:],
                        start=(tpp == 0), stop=(tpp == NP - 1),
                        perf_mode=DR,
                    )
                # one final-projection pair per slot once chunks are queued
                elif pending and nfin < 1 and tp >= 1:
                    emit_final_pair("big")
                    nfin += 1

            def _rec(gi=gi, acc=acc):
                rec = nrm.tile([D, 512], F32, tag="rec", bufs=2,
                               name=f"rec{gi}")
                nc.vector.reciprocal_approx_fast(rec[:], acc[0:D, :])
                recs[0] = rec

            def _oz(m=m, h2=h2, sb=sb, dbase=dbase, ssl=ssl, acc=acc):
                nc.vector.tensor_tensor(
                    out=ozb[dbase:dbase + D, m, ssl],
                    in0=acc[D:2 * D, :], in1=recs[0][:],
                    op=ALU.mult,
                )
                if m == 1 and h2 == 1:
                    pending.extend((mo, sb) for mo in range(0, E // P, 2))

            deferred.extend([_rec, _oz])

        while deferred:
            deferred.pop(0)()
        # drain: remaining projection pairs reuse the freed big psum tag;
        # copies strictly alternate engines so they run two-wide
        di = 0
        while pending:
            emit_final_pair("big", engine="a" if di % 2 == 0 else "d")
            di += 1


def _build_body(tc, xT8, w8, wc16d, yT, dbg=None):
    nc = tc.nc

    with (
        tc.tile_pool(name="const", bufs=1) as const,
        tc.tile_pool(name="wc", bufs=1) as wcp,
        tc.tile_pool(name="vz", bufs=1) as vzp,
        tc.tile_pool(name="zT", bufs=1) as zTp,
        tc.tile_pool(name="zs", bufs=1) as zsp,
        tc.tile_pool(name="cdz", bufs=1) as cdzp,
        tc.tile_pool(name="ps", bufs=1, space="PSUM") as ps,
    ):
        hp = const.tile([P, 1], F32)
        nc.vector.memset(hp[:], HALF_PI)
        warm = const.tile([P, 512], F32)
        nc.vector.memset(warm[:], 0.0)
        wc16 = wcp.tile([P, 2, E], F16)

        # vz pair tiles: [t-in-pair, head, (ones|z), 64]
        vz = [vzp.tile([P, 2, HG, 2, D], F8, tag=f"vz{j}", name=f"vz{j}")
              for j in range(NT // 2)]

        # zqkT per-sb buffers: [chunk(zq m0, zq m1, zk m0, zk m1), t', s]
        zqkT = [zTp.tile([P, 4, 4, P], F16, tag=f"zqkT{sb}", name=f"zqkT{sb}")
                for sb in range(4)]

        zqs = zsp.tile([P, NT, 3 * EG], F16, tag="zqs", name="zqs")[:]

        c_bufs = [cdzp.tile([P, 3 * EG], F16, tag=f"c{i}", name=f"c{i}")
                  for i in range(3)]
        dz_bufs = [cdzp.tile([P, 3 * EG], F16, tag=f"dz{i}", name=f"dz{i}")
                   for i in range(3)]
        for i in range(3):
            # c segment-boundary cols must stay 0 (reset trick); dz must be
            # 0 outside the boundary cols. These gate the first scans, so
            # they precede the vz ones-memsets on Pool.
            nc.gpsimd.memset(
                c_bufs[i][:].rearrange("p (s c) -> p s c", c=D)[:, :, 0:1],
                0.0)
            nc.gpsimd.memset(dz_bufs[i][:], 0.0)
        for j in range(NT // 2):
            nc.gpsimd.memset(vz[j][:, :, :, 0, :], 1.0)

        # ---------------- input DMA + phase A ----------------
        with (
            tc.tile_pool(name="x", bufs=1) as xp,
            tc.tile_pool(name="w", bufs=1) as wp,
        ):
            x8 = xp.tile([P, KC, S], F8, tag="x8", name="x8")
            w8s = wp.tile([P, KC, 3 * EG], F8, tag="w8", name="w8s")
            xTr = xT8.rearrange("(k p) s -> p k s", p=P)
            wTr = w8.rearrange("(k p) n -> p k n", p=P)
            # few big DMAs: HWDGE slot cost (625 ns) dominates small ones.
            # x lands s-chunk-major (all k per 512 s-cols) so tile t's
            # theta completes right after chunk t//4 -- the first Sin can
            # start ~4 us in instead of waiting the whole x transfer.
            nc.sync.dma_start(out=w8s[:], in_=wTr[:])
            for sb4 in range(4):
                ssl4 = slice(sb4 * 512, (sb4 + 1) * 512)
                nc.sync.dma_start(out=x8[:, :, ssl4], in_=xTr[:, :, ssl4])
            nc.sync.dma_start(
                out=wc16[:],
                in_=wc16d.rearrange("(m p) e -> p m e", p=P))

            _phase_a(tc, x8[:], w8s[:], hp, warm, c_bufs, dz_bufs, zqs, zqkT,
                     vz, ps)

            if _DEBUG:
                nc.sync.dma_start(out=dbg["dbg_c0"][:], in_=c_bufs[0][:])
                nc.sync.dma_start(out=dbg["dbg_zqs0"][:], in_=zqs[:, 0, :])
                nc.sync.dma_start(
                    out=dbg["dbg_zqkT0"][:],
                    in_=zqkT[0][:].rearrange("p a b c -> p (a b c)"))
                nc.sync.dma_start(
                    out=dbg["dbg_vz0"][:],
                    in_=vz[0][:].rearrange("p a h s d -> p (a h s d)"))

            # ---------------- phase B ----------------
            with tc.tile_pool(name="oz", bufs=1) as ozp:
                ozb = ozp.tile([P, 2, S], F16, tag="ozb", name="ozb")[:]
                _attention(tc, ozb, zqkT, vz, wc16[:], yT, ps, dbg)
                if _DEBUG:
                    nc.sync.dma_start(
                        out=dbg["dbg_oz0"][:],
                        in_=ozb[:, 0, :])


def build_bass():
    nc = bacc.Bacc(None, target_bir_lowering=False)
    xT8 = nc.dram_tensor("xT8", [E, S], F8, kind="ExternalInput")
    w8 = nc.dram_tensor("w8", [E, 3 * EG], F8, kind="ExternalInput")
    wc16d = nc.dram_tensor("wc16d", [EG, E], F16, kind="ExternalInput")
    yT = nc.dram_tensor("yT", [E, S], F16, kind="ExternalOutput")
    dbg = {}
    if _DEBUG:
        for nm, shp, dt in (("dbg_c0", [P, 3 * EG], F16),
                            ("dbg_zqs0", [P, 3 * EG], F16),
                            ("dbg_zqkT0", [P, 4 * 4 * P], F16),
                            ("dbg_vz0", [P, 2 * HG * 2 * D], F8),
                            ("dbg_ex0", [P, 2, 512], F8),
                            ("dbg_oz0", [P, S], F8)):
            dbg[nm] = nc.dram_tensor(nm, shp, dt, kind="ExternalOutput")[:]
    with tile.TileContext(nc) as tc:
        _build_body(tc, xT8[:], w8[:], wc16d[:], yT[:], dbg)
    nc.finalize()
    return nc


_NC_CACHE = None


def _get_nc():
    global _NC_CACHE
    if _NC_CACHE is None:
        _NC_CACHE = build_bass()
    return _NC_CACHE


def kernel(x, Wq, Wk, Wv, Wc, bc, **kw):
    import ml_dtypes
    f8 = ml_dtypes.float8_e4m3
    x = np.asarray(x, np.float32)
    in_maps = []
    for c in range(NCORES):
        b, g = divmod(c, NCORES // B)
        sl = slice(g * EG, (g + 1) * EG)
        wqkv = np.concatenate(
            [np.asarray(Wq)[sl, :].T, np.asarray(Wk)[sl, :].T,
             np.asarray(Wv)[sl, :].T], axis=1) * WSCALE
        in_maps.append({
            "xT8": np.ascontiguousarray(x[b].T).astype(f8),
            "w8": np.ascontiguousarray(wqkv).astype(f8),
            "wc16d": np.ascontiguousarray(
                np.asarray(Wc)[:, sl].T).astype(np.float16),
        })
    nc = _get_nc()
    res = run_bass_kernel_spmd(
        nc, in_maps, core_ids=list(range(NCORES)),
        trace=bool(int(os.environ.get("QK_TRACE", "0"))),
    )
    y = np.zeros((B, S, E), np.float32)
    for c in range(NCORES):
        b = c // (NCORES // B)
        y[b] += res.results[c]["yT"].astype(np.float32).T
    y += np.asarray(bc, np.float32)
    globals()["_LAST_RESULT"] = res
    return y


# revision 34
# speedup vs baseline: 1.0294x; 1.0294x over previous
"""Trainium2 Bass kernel: multi-head attention with quantum (cumprod-of-cos)
transform.

Full-input contract: kernel(**inputs) takes the unsharded inputs and returns
the full [B, S, E] output. Internally shards over 8 NeuronCores: data-parallel
over batch (B=2) x tensor-parallel over head-groups (4 heads per core).

v2 design (fp8 DoubleRow + engine rebalance). Per core:

Phase A, per 128-row s-tile t (16 tiles):
  - theta = x @ [Wq|Wk|Wv]^T via fp8e4 DoubleRow matmuls (x, 64*W in fp8;
    DR packs 2 contraction k-tiles per instruction at 0.5 cycles/row -> 4x
    fewer PE cycles than fp16). th psum tile [128, 2, 512] (bank0 = q|k,
    bank1 = v).
  - cos via Sin activation (bias pi/2, scale 1/64 compensating the weight
    prescale). c layout = 12 segments x 64 (q0..3, k0..3, v0..3) where each
    segment's col 0 stays ZERO in c; the boundary cos value goes to dz
    (second activation, 12 cols). This enables ONE segmented cumprod scan:
      state = c*state + dz  -> at seg start: 0*state + cos0 (reset);
    one DVE tensor_tensor_scan [128, 768] per tile instead of 12.
  - q/k scan results (fp16) are transposed by the DMA engines
    (dma_start_transpose, idle XBAR hardware) into per-sb buffers
    zqkT[sb][128, 4chunk, 4t', 128] -- no PE transposes, no psum->sbuf
    copies. v results convert fp16->fp8 on Pool (tensor_copy) into the
    vz pair tiles [128, 2t, 4h, (ones|z), 64]; ones slots are memset once.

Phase B attention (two interleaved (m, h2) streams, s-chunk-major), per
t-PAIR (8 steps per stream):
  - 2 score matmuls (fp16, K=64, tile_position row-halves) into a 2-bank
    sc2 psum tile [128, 2, 512]
  - ONE exp over the pair [128, 1024] -> ex fp8e4 [128, 2, 512]:
    alternating ACT (activation Exp, scale=1/8) and DVE (int8 Schraudolph:
    i8 = A*s + B bitcast fp8e4) to balance the two engines
  - ONE fp8 DoubleRow acc matmul per pair: lhsT = vz[ones|z] (denominator
    free via ones), rhs = ex pair; 8 DR matmuls accumulate all 16 t-tiles
  - rec merged over both streams (reciprocal_approx_fast [64, 2, 512]);
    oz = acc[64:128]*rec -> fp8 into ozb [128, 2m, S]
  - final projection: ONE fp8 DR matmul per 128-row out chunk (Wc pair
    packed), psum pairs copied fp16 (ACT/DVE alternating) and DMA'd out.

Single PSUM pool, tags "big" ([128,2,512] = th in A, sc2/py2 in B, 2 bufs)
and "acc" (2 bufs) = exactly 8 banks; no pool-transition barrier, so phase B
score matmuls overlap phase A's tail.

Weights are prescaled by 64 on host (fp8e4m3 normal range); y is divided by
64 on host. ACT tables: Sin (A) + Exp (B), 2 loads.
"""

import os
import sys

import numpy as np

if "/opt/trn_rl_repo" not in sys.path:
    sys.path.insert(0, "/opt/trn_rl_repo")

import concourse.bass as bass  # noqa: F401
import concourse.tile as tile
from concourse import bacc
from concourse import mybir
from concourse.bass_utils import run_bass_kernel_spmd

AF = mybir.ActivationFunctionType
ALU = mybir.AluOpType
F32 = mybir.dt.float32
F16 = mybir.dt.float16
F8 = mybir.dt.float8e4
I8 = mybir.dt.int8
DR = mybir.MatmulPerfMode.DoubleRow

B, S, E, H, D = 2, 2048, 1024, 16, 64
NCORES = 8
HG = 4          # heads per core
EG = HG * D     # 256
P = 128
NT = S // P     # 16 s-tiles
KC = E // P     # 8 contraction tiles for the projections
HALF_PI = float(np.pi / 2)
INV_SQRT_D = 0.125  # 1/sqrt(64)
WSCALE = 64.0       # host-side weight prescale so fp8e4m3 stays normal

# fp8e4m3 Schraudolph: exp(s/8) ~= bitcast_e4m3(i8(EXP_A*s + EXP_B));
# max rel err ~7.3%, mean 2.6% (calibrated against the executor's
# fp32->int8 truncation).
EXP_A = float(np.log2(np.e))  # 1.4427: 8 * log2(e) * (1/8)
EXP_B = 56.13
# engine pattern for the per-pair exp: ACT-heavy to balance DVE's scan +
# rec/oz load. idx % EXP_MOD in EXP_DVE -> DVE Schraudolph, else ACT.
EXP_MOD = 8
EXP_DVE = (1, 3, 6)

_DEBUG = bool(int(os.environ.get("QK_DEBUG", "0")))


def _phase_a(tc, x8, w8s, hp, warm, c_bufs, dz_bufs, zqs, zqkT, vz, ps):
    nc = tc.nc
    # PE p-state warm-up: ~6 us of dep-free fp32 matmuls during the DMA
    # lead-in so the theta matmuls run at the full 2.4 GHz p-state.
    wps = ps.tile([P, 2, 512], F32, tag="big", bufs=3, name="warmps")
    for wi in range(4):
        nc.tensor.matmul(wps[0:1, 0, :], lhsT=warm[:, 0:1], rhs=warm[:],
                         start=True, stop=True, skip_group_check=True)
    for t in range(NT):
        tsl = slice(t * P, (t + 1) * P)
        th = ps.tile([P, 2, 512], F32, tag="big", bufs=3, name=f"th{t}")
        for j in range(4):
            nc.tensor.matmul(
                th[:, 0, :],
                lhsT=x8[:, 2 * j:2 * j + 2, tsl],
                rhs=w8s[:, 2 * j:2 * j + 2, 0:2 * EG],
                start=(j == 0), stop=(j == 3),
                perf_mode=DR,
            )
            nc.tensor.matmul(
                th[:, 1, 0:EG],
                lhsT=x8[:, 2 * j:2 * j + 2, tsl],
                rhs=w8s[:, 2 * j:2 * j + 2, 2 * EG:3 * EG],
                start=(j == 0), stop=(j == 3),
                perf_mode=DR,
            )
        c = c_bufs[t % len(c_bufs)]
        dz = dz_bufs[t % len(dz_bufs)]
        th_seg = th[:].rearrange("p a b -> p (a b)").rearrange(
            "p (s c) -> p s c", c=D)
        c_seg = c[:].rearrange("p (s c) -> p s c", c=D)
        # main Sin: cols 1..63 of each of the 12 segments (v segs start at
        # flat col 512 in th, which is exactly seg index 8 -- uniform stride)
        nc.scalar.activation(
            c_seg[:, 0:12, 1:D], th_seg[:, 0:12, 1:D], AF.Sin,
            bias=hp[:], scale=1.0 / WSCALE)
        # boundary Sin: col 0 of each segment -> dz (c col 0 stays zero)
        dz_seg = dz[:].rearrange("p (s c) -> p s c", c=D)
        nc.scalar.activation(
            dz_seg[:, 0:12, 0:1], th_seg[:, 0:12, 0:1], AF.Sin,
            bias=hp[:], scale=1.0 / WSCALE)
        # segmented cumprod: state = c*state + dz (reset at seg starts)
        nc.vector.tensor_tensor_scan(
            zqs[:, t, :], c[:], dz[:], 0.0, ALU.mult, ALU.add)
        # v part -> fp8 into the vz pair tile (Pool)
        nc.gpsimd.tensor_copy(
            out=vz[t // 2][:, t % 2, :, 1, :],
            in_=zqs[:, t, 2 * EG:3 * EG].rearrange("p (h d) -> p h d", d=D))
        # q+k -> transposed via DMA xbar into the per-sb buffer
        nc.sync.dma_start_transpose(
            out=zqkT[t // 4][:, :, t % 4, :],
            in_=zqs[:, t, 0:2 * EG])


def _attention(tc, ozb, zqkT, vz, wc16, yT, ps, dbg=None):
    nc = tc.nc
    ex_idx = 0
    cp_idx = 0
    pending = []  # (mo, sb) final-projection chunk pairs ready to project

    with (
        tc.tile_pool(name="y", bufs=3) as yp,
        tc.tile_pool(name="norm", bufs=2) as nrm,
        tc.tile_pool(name="exps", bufs=12) as exq,
    ):
        def emit_final_pair(tag, engine=None):
            nonlocal cp_idx
            mo, sb = pending.pop(0)
            ssl = slice(sb * 512, (sb + 1) * 512)
            py2 = ps.tile([P, 2, 512], F32, tag=tag, bufs=3,
                          name=f"py{mo}_{sb}")
            for i in range(2):
                for kk in range(2):
                    nc.tensor.matmul(
                        py2[:, i, :],
                        lhsT=wc16[:, kk, (mo + i) * P:(mo + i + 1) * P],
                        rhs=ozb[:, kk, ssl],
                        start=(kk == 0), stop=(kk == 1),
                    )
            yt = yp.tile([P, 2, 512], F16, tag="y", name=f"yt{mo}_{sb}")
            if engine is None:
                engine = "a" if cp_idx % 2 == 0 else "d"
            if engine == "a":
                nc.scalar.copy(
                    out=yt[:], in_=py2[:].rearrange("p a b -> p (a b)"))
            else:
                nc.vector.tensor_copy(
                    out=yt[:], in_=py2[:].rearrange("p a b -> p (a b)"))
            cp_idx += 1
            nc.sync.dma_start(
                out=yT[mo * P:(mo + 2) * P, ssl].rearrange(
                    "(k p) s -> p k s", p=P),
                in_=yt[:])

        NP = NT // 2  # 8 t-pairs
        combos = [(m, h2, sb) for sb in range(4) for m in range(2)
                  for h2 in range(2)]
        # One stream per group: acc is a single 1-bank tile (bufs=2, so
        # group boundaries never WAR-block), and sc2's 3 buffers serve ONE
        # allocation per slot -- a 3-slot lookahead that covers the
        # score->exp->release round trip (~1.8us) at ~0.9us slots.
        LAG = 2
        recs = [None]
        deferred = []  # prev group's rec/oz closures, one per slot
        for gi, (m, h2, sb) in enumerate(combos):
            h = 2 * m + h2
            dbase = h2 * D
            ssl = slice(sb * 512, (sb + 1) * 512)
            acc = ps.tile([P, 512], F32, tag="acc", bufs=2, name=f"acc{gi}")
            exs = [None] * NP
            nfin = 0
            for tp in range(NP + LAG):
                # prev group's rec/oz, one per slot so they interleave
                # between this group's DVE exps instead of damming them
                if deferred and tp >= 1:
                    deferred.pop(0)()
                if tp < NP:
                    sc2 = ps.tile([P, 2, 512], F32, tag="big", bufs=3,
                                  name=f"sc{gi}_{tp}")
                    for tt in range(2):
                        t = 2 * tp + tt
                        nc.tensor.matmul(
                            sc2[:, tt, :],
                            lhsT=zqkT[t // 4][dbase:dbase + D, 2 + m,
                                              t % 4, :],
                            rhs=zqkT[sb][dbase:dbase + D, m, :, :]
                                .rearrange("p a b -> p (a b)"),
                            start=True, stop=True,
                            tile_position=(dbase, 0),
                        )
                    ex = exq.tile([P, 2, 512], F8, tag="ex",
                                  name=f"ex{gi}_{tp}")
                    sc_flat = sc2[:].rearrange("p a b -> p (a b)")
                    if (ex_idx % EXP_MOD) in EXP_DVE:
                        nc.vector.tensor_scalar(
                            out=ex[:].rearrange(
                                "p a b -> p (a b)").bitcast(I8),
                            in0=sc_flat,
                            scalar1=EXP_A, scalar2=EXP_B,
                            op0=ALU.mult, op1=ALU.add,
                        )
                    else:
                        nc.scalar.activation(
                            ex[:].rearrange("p a b -> p (a b)"),
                            sc_flat, AF.Exp, scale=INV_SQRT_D)
                    if _DEBUG and gi == 0 and tp == 0:
                        nc.sync.dma_start(out=dbg["dbg_ex0"][:], in_=ex[:])
                    ex_idx += 1
                    exs[tp] = ex
                if tp >= LAG:
                    tpp = tp - LAG
                    nc.tensor.matmul(
                        acc[:],
                        lhsT=vz[tpp][:, :, h, :, :].rearrange(
                            "p a b c -> p a (b c)"),
                        rhs=exs[tpp][:],
                        start=(tpp == 0), stop=(tpp == NP - 1),
                        perf_mode=DR,
                    )
                # one final-projection pair per slot once chunks are queued
                elif pending and nfin < 1 and tp >= 1:
                    emit_final_pair("big")
                    nfin += 1

            def _rec(gi=gi, acc=acc):
                rec = nrm.tile([D, 512], F32, tag="rec", bufs=2,
                               name=f"rec{gi}")
                nc.vector.reciprocal_approx_fast(rec[:], acc[0:D, :])
                recs[0] = rec

            def _oz(m=m, h2=h2, sb=sb, dbase=dbase, ssl=ssl, acc=acc):
                nc.vector.tensor_tensor(
                    out=ozb[dbase:dbase + D, m, ssl],
                    in0=acc[D:2 * D, :], in1=recs[0][:],
                    op=ALU.mult,
                )
                if m == 1 and h2 == 1:
                    pending.extend((mo, sb) for mo in range(0, E // P, 2))

            deferred.extend([_rec, _oz])

        while deferred:
            deferred.pop(0)()
        # drain: remaining projection pairs reuse the freed big psum tag;
        # copies strictly alternate engines so they run two-wide
        di = 0
        while pending:
            emit_final_pair("big", engine="a" if di % 2 == 0 else "d")
            di += 1


def _build_body(tc, xT8, w8, wc16d, yT, dbg=None):
    nc = tc.nc

    with (
        tc.tile_pool(name="const", bufs=1) as const,
        tc.tile_pool(name="wc", bufs=1) as wcp,
        tc.tile_pool(name="vz", bufs=1) as vzp,
        tc.tile_pool(name="zT", bufs=1) as zTp,
        tc.tile_pool(name="zs", bufs=1) as zsp,
        tc.tile_pool(name="cdz", bufs=1) as cdzp,
        tc.tile_pool(name="ps", bufs=1, space="PSUM") as ps,
    ):
        hp = const.tile([P, 1], F32)
        nc.vector.memset(hp[:], HALF_PI)
        warm = const.tile([P, 512], F32)
        nc.vector.memset(warm[:], 0.0)
        wc16 = wcp.tile([P, 2, E], F16)

        # vz pair tiles: [t-in-pair, head, (ones|z), 64]
        vz = [vzp.tile([P, 2, HG, 2, D], F8, tag=f"vz{j}", name=f"vz{j}")
              for j in range(NT // 2)]

        # zqkT per-sb buffers: [chunk(zq m0, zq m1, zk m0, zk m1), t', s]
        zqkT = [zTp.tile([P, 4, 4, P], F16, tag=f"zqkT{sb}", name=f"zqkT{sb}")
                for sb in range(4)]

        zqs = zsp.tile([P, NT, 3 * EG], F16, tag="zqs", name="zqs")[:]

        c_bufs = [cdzp.tile([P, 3 * EG], F16, tag=f"c{i}", name=f"c{i}")
                  for i in range(3)]
        dz_bufs = [cdzp.tile([P, 3 * EG], F16, tag=f"dz{i}", name=f"dz{i}")
                   for i in range(3)]
        for i in range(3):
            # c segment-boundary cols must stay 0 (reset trick); dz must be
            # 0 outside the boundary cols. These gate the first scans, so
            # they precede the vz ones-memsets on Pool.
            nc.gpsimd.memset(
                c_bufs[i][:].rearrange("p (s c) -> p s c", c=D)[:, :, 0:1],
                0.0)
            nc.gpsimd.memset(dz_bufs[i][:], 0.0)
        for j in range(NT // 2):
            nc.gpsimd.memset(vz[j][:, :, :, 0, :], 1.0)

        # ---------------- input DMA + phase A ----------------
        with (
            tc.tile_pool(name="x", bufs=1) as xp,
            tc.tile_pool(name="w", bufs=1) as wp,
        ):
            x8 = xp.tile([P, KC, S], F8, tag="x8", name="x8")
            w8s = wp.tile([P, KC, 3 * EG], F8, tag="w8", name="w8s")
            xTr = xT8.rearrange("(k p) s -> p k s", p=P)
            wTr = w8.rearrange("(k p) n -> p k n", p=P)
            # few big DMAs: HWDGE slot cost (625 ns) dominates small ones.
            # x lands s-chunk-major (all k per 512 s-cols) so tile t's
            # theta completes right after chunk t//4 -- the first Sin can
            # start ~4 us in instead of waiting the whole x transfer.
            nc.sync.dma_start(out=w8s[:], in_=wTr[:])
            for sb4 in range(4):
                ssl4 = slice(sb4 * 512, (sb4 + 1) * 512)
                nc.sync.dma_start(out=x8[:, :, ssl4], in_=xTr[:, :, ssl4])
            nc.sync.dma_start(
                out=wc16[:],
                in_=wc16d.rearrange("(m p) e -> p m e", p=P))

            _phase_a(tc, x8[:], w8s[:], hp, warm, c_bufs, dz_bufs, zqs, zqkT,
                     vz, ps)

            if _DEBUG:
                nc.sync.dma_start(out=dbg["dbg_c0"][:], in_=c_bufs[0][:])
                nc.sync.dma_start(out=dbg["dbg_zqs0"][:], in_=zqs[:, 0, :])
                nc.sync.dma_start(
                    out=dbg["dbg_zqkT0"][:],
                    in_=zqkT[0][:].rearrange("p a b c -> p (a b c)"))
                nc.sync.dma_start(
                    out=dbg["dbg_vz0"][:],
                    in_=vz[0][:].rearrange("p a h s d -> p (a h s d)"))

            # ---------------- phase B ----------------
            with tc.tile_pool(name="oz", bufs=1) as ozp:
                ozb = ozp.tile([P, 2, S], F16, tag="ozb", name="ozb")[:]
                _attention(tc, ozb, zqkT, vz, wc16[:], yT, ps, dbg)
                if _DEBUG:
                    nc.sync.dma_start(
                        out=dbg["dbg_oz0"][:],
                        in_=ozb[:, 0, :])


def build_bass():
    nc = bacc.Bacc(None, target_bir_lowering=False)
    xT8 = nc.dram_tensor("xT8", [E, S], F8, kind="ExternalInput")
    w8 = nc.dram_tensor("w8", [E, 3 * EG], F8, kind="ExternalInput")
    wc16d = nc.dram_tensor("wc16d", [EG, E], F16, kind="ExternalInput")
    yT = nc.dram_tensor("yT", [E, S], F16, kind="ExternalOutput")
    dbg = {}
    if _DEBUG:
        for nm, shp, dt in (("dbg_c0", [P, 3 * EG], F16),
                            ("dbg_zqs0", [P, 3 * EG], F16),
                            ("dbg_zqkT0", [P, 4 * 4 * P], F16),
                            ("dbg_vz0", [P, 2 * HG * 2 * D], F8),
                            ("dbg_ex0", [P, 2, 512], F8),
                            ("dbg_oz0", [P, S], F8)):
            dbg[nm] = nc.dram_tensor(nm, shp, dt, kind="ExternalOutput")[:]
    with tile.TileContext(nc) as tc:
        _build_body(tc, xT8[:], w8[:], wc16d[:], yT[:], dbg)
    nc.finalize()
    return nc


_NC_CACHE = None


def _get_nc():
    global _NC_CACHE
    if _NC_CACHE is None:
        _NC_CACHE = build_bass()
    return _NC_CACHE


def kernel(x, Wq, Wk, Wv, Wc, bc, **kw):
    import ml_dtypes
    f8 = ml_dtypes.float8_e4m3
    x = np.asarray(x, np.float32)
    in_maps = []
    for c in range(NCORES):
        b, g = divmod(c, NCORES // B)
        sl = slice(g * EG, (g + 1) * EG)
        wqkv = np.concatenate(
            [np.asarray(Wq)[sl, :].T, np.asarray(Wk)[sl, :].T,
             np.asarray(Wv)[sl, :].T], axis=1) * WSCALE
        in_maps.append({
            "xT8": np.ascontiguousarray(x[b].T).astype(f8),
            "w8": np.ascontiguousarray(wqkv).astype(f8),
            "wc16d": np.ascontiguousarray(
                np.asarray(Wc)[:, sl].T).astype(np.float16),
        })
    nc = _get_nc()
    res = run_bass_kernel_spmd(
        nc, in_maps, core_ids=list(range(NCORES)),
        trace=bool(int(os.environ.get("QK_TRACE", "0"))),
    )
    y = np.zeros((B, S, E), np.float32)
    for c in range(NCORES):
        b = c // (NCORES // B)
        y[b] += res.results[c]["yT"].astype(np.float32).T
    y += np.asarray(bc, np.float32)
    globals()["_LAST_RESULT"] = res
    return y


# revision 35
# speedup vs baseline: 1.0413x; 1.0116x over previous
"""Trainium2 Bass kernel: multi-head attention with quantum (cumprod-of-cos)
transform.

Full-input contract: kernel(**inputs) takes the unsharded inputs and returns
the full [B, S, E] output. Internally shards over 8 NeuronCores: data-parallel
over batch (B=2) x tensor-parallel over head-groups (4 heads per core).

v2 design (fp8 DoubleRow + engine rebalance). Per core:

Phase A, per 128-row s-tile t (16 tiles):
  - theta = x @ [Wq|Wk|Wv]^T via fp8e4 DoubleRow matmuls (x, 64*W in fp8;
    DR packs 2 contraction k-tiles per instruction at 0.5 cycles/row -> 4x
    fewer PE cycles than fp16). th psum tile [128, 2, 512] (bank0 = q|k,
    bank1 = v).
  - cos via Sin activation (bias pi/2, scale 1/64 compensating the weight
    prescale). c layout = 12 segments x 64 (q0..3, k0..3, v0..3) where each
    segment's col 0 stays ZERO in c; the boundary cos value goes to dz
    (second activation, 12 cols). This enables ONE segmented cumprod scan:
      state = c*state + dz  -> at seg start: 0*state + cos0 (reset);
    one DVE tensor_tensor_scan [128, 768] per tile instead of 12.
  - q/k scan results (fp16) are transposed by the DMA engines
    (dma_start_transpose, idle XBAR hardware) into per-sb buffers
    zqkT[sb][128, 4chunk, 4t', 128] -- no PE transposes, no psum->sbuf
    copies. v results convert fp16->fp8 on Pool (tensor_copy) into the
    vz pair tiles [128, 2t, 4h, (ones|z), 64]; ones slots are memset once.

Phase B attention (two interleaved (m, h2) streams, s-chunk-major), per
t-PAIR (8 steps per stream):
  - 2 score matmuls (fp16, K=64, tile_position row-halves) into a 2-bank
    sc2 psum tile [128, 2, 512]
  - ONE exp over the pair [128, 1024] -> ex fp8e4 [128, 2, 512]:
    alternating ACT (activation Exp, scale=1/8) and DVE (int8 Schraudolph:
    i8 = A*s + B bitcast fp8e4) to balance the two engines
  - ONE fp8 DoubleRow acc matmul per pair: lhsT = vz[ones|z] (denominator
    free via ones), rhs = ex pair; 8 DR matmuls accumulate all 16 t-tiles
  - rec merged over both streams (reciprocal_approx_fast [64, 2, 512]);
    oz = acc[64:128]*rec -> fp8 into ozb [128, 2m, S]
  - final projection: ONE fp8 DR matmul per 128-row out chunk (Wc pair
    packed), psum pairs copied fp16 (ACT/DVE alternating) and DMA'd out.

Single PSUM pool, tags "big" ([128,2,512] = th in A, sc2/py2 in B, 2 bufs)
and "acc" (2 bufs) = exactly 8 banks; no pool-transition barrier, so phase B
score matmuls overlap phase A's tail.

Weights are prescaled by 64 on host (fp8e4m3 normal range); y is divided by
64 on host. ACT tables: Sin (A) + Exp (B), 2 loads.
"""

import os
import sys

import numpy as np

if "/opt/trn_rl_repo" not in sys.path:
    sys.path.insert(0, "/opt/trn_rl_repo")

import concourse.bass as bass  # noqa: F401
import concourse.tile as tile
from concourse import bacc
from concourse import mybir
from concourse.bass_utils import run_bass_kernel_spmd

AF = mybir.ActivationFunctionType
ALU = mybir.AluOpType
F32 = mybir.dt.float32
F16 = mybir.dt.float16
F8 = mybir.dt.float8e4
I8 = mybir.dt.int8
DR = mybir.MatmulPerfMode.DoubleRow

B, S, E, H, D = 2, 2048, 1024, 16, 64
NCORES = 8
HG = 4          # heads per core
EG = HG * D     # 256
P = 128
NT = S // P     # 16 s-tiles
KC = E // P     # 8 contraction tiles for the projections
HALF_PI = float(np.pi / 2)
INV_SQRT_D = 0.125  # 1/sqrt(64)
WSCALE = 64.0       # host-side weight prescale so fp8e4m3 stays normal

# fp8e4m3 Schraudolph: exp(s/8) ~= bitcast_e4m3(i8(EXP_A*s + EXP_B));
# max rel err ~7.3%, mean 2.6% (calibrated against the executor's
# fp32->int8 truncation).
EXP_A = float(np.log2(np.e))  # 1.4427: 8 * log2(e) * (1/8)
EXP_B = 56.13
# engine pattern for the per-pair exp: ACT-heavy to balance DVE's scan +
# rec/oz load. idx % EXP_MOD in EXP_DVE -> DVE Schraudolph, else ACT.
EXP_MOD = 8
EXP_DVE = (1, 4, 6)

_DEBUG = bool(int(os.environ.get("QK_DEBUG", "0")))


def _phase_a(tc, x8, w8s, hp, warm, c_bufs, dz_bufs, zqs, zqkT, vz, ps):
    nc = tc.nc
    # PE p-state warm-up: ~6 us of dep-free fp32 matmuls during the DMA
    # lead-in so the theta matmuls run at the full 2.4 GHz p-state.
    wps = ps.tile([P, 2, 512], F32, tag="big", bufs=3, name="warmps")
    for wi in range(4):
        nc.tensor.matmul(wps[0:1, 0, :], lhsT=warm[:, 0:1], rhs=warm[:],
                         start=True, stop=True, skip_group_check=True)
    for t in range(NT):
        tsl = slice(t * P, (t + 1) * P)
        th = ps.tile([P, 2, 512], F32, tag="big", bufs=3, name=f"th{t}")
        for j in range(4):
            nc.tensor.matmul(
                th[:, 0, :],
                lhsT=x8[:, 2 * j:2 * j + 2, tsl],
                rhs=w8s[:, 2 * j:2 * j + 2, 0:2 * EG],
                start=(j == 0), stop=(j == 3),
                perf_mode=DR,
            )
            nc.tensor.matmul(
                th[:, 1, 0:EG],
                lhsT=x8[:, 2 * j:2 * j + 2, tsl],
                rhs=w8s[:, 2 * j:2 * j + 2, 2 * EG:3 * EG],
                start=(j == 0), stop=(j == 3),
                perf_mode=DR,
            )
        c = c_bufs[t % len(c_bufs)]
        dz = dz_bufs[t % len(dz_bufs)]
        th_seg = th[:].rearrange("p a b -> p (a b)").rearrange(
            "p (s c) -> p s c", c=D)
        c_seg = c[:].rearrange("p (s c) -> p s c", c=D)
        # main Sin: cols 1..63 of each of the 12 segments (v segs start at
        # flat col 512 in th, which is exactly seg index 8 -- uniform stride)
        nc.scalar.activation(
            c_seg[:, 0:12, 1:D], th_seg[:, 0:12, 1:D], AF.Sin,
            bias=hp[:], scale=1.0 / WSCALE)
        # boundary Sin: col 0 of each segment -> dz (c col 0 stays zero)
        dz_seg = dz[:].rearrange("p (s c) -> p s c", c=D)
        nc.scalar.activation(
            dz_seg[:, 0:12, 0:1], th_seg[:, 0:12, 0:1], AF.Sin,
            bias=hp[:], scale=1.0 / WSCALE)
        # segmented cumprod: state = c*state + dz (reset at seg starts)
        nc.vector.tensor_tensor_scan(
            zqs[:, t, :], c[:], dz[:], 0.0, ALU.mult, ALU.add)
        # v part -> fp8 into the vz pair tile (Pool)
        nc.gpsimd.tensor_copy(
            out=vz[t // 2][:, t % 2, :, 1, :],
            in_=zqs[:, t, 2 * EG:3 * EG].rearrange("p (h d) -> p h d", d=D))
        # q+k -> transposed via DMA xbar into the per-sb buffer
        nc.sync.dma_start_transpose(
            out=zqkT[t // 4][:, :, t % 4, :],
            in_=zqs[:, t, 0:2 * EG])


def _attention(tc, ozb, zqkT, vz, wc16, yT, ps, dbg=None):
    nc = tc.nc
    ex_idx = 0
    cp_idx = 0
    pending = []  # (mo, sb) final-projection chunk pairs ready to project

    with (
        tc.tile_pool(name="y", bufs=3) as yp,
        tc.tile_pool(name="norm", bufs=2) as nrm,
        tc.tile_pool(name="exps", bufs=12) as exq,
    ):
        def emit_final_pair(tag, engine=None):
            nonlocal cp_idx
            mo, sb = pending.pop(0)
            ssl = slice(sb * 512, (sb + 1) * 512)
            py2 = ps.tile([P, 2, 512], F32, tag=tag, bufs=3,
                          name=f"py{mo}_{sb}")
            for i in range(2):
                for kk in range(2):
                    nc.tensor.matmul(
                        py2[:, i, :],
                        lhsT=wc16[:, kk, (mo + i) * P:(mo + i + 1) * P],
                        rhs=ozb[:, kk, ssl],
                        start=(kk == 0), stop=(kk == 1),
                    )
            yt = yp.tile([P, 2, 512], F16, tag="y", name=f"yt{mo}_{sb}")
            if engine is None:
                engine = "a" if cp_idx % 2 == 0 else "d"
            if engine == "a":
                nc.scalar.copy(
                    out=yt[:], in_=py2[:].rearrange("p a b -> p (a b)"))
            else:
                nc.vector.tensor_copy(
                    out=yt[:], in_=py2[:].rearrange("p a b -> p (a b)"))
            cp_idx += 1
            nc.sync.dma_start(
                out=yT[mo * P:(mo + 2) * P, ssl].rearrange(
                    "(k p) s -> p k s", p=P),
                in_=yt[:])

        NP = NT // 2  # 8 t-pairs
        combos = [(m, h2, sb) for sb in range(4) for m in range(2)
                  for h2 in range(2)]
        # One stream per group: acc is a single 1-bank tile (bufs=2, so
        # group boundaries never WAR-block), and sc2's 3 buffers serve ONE
        # allocation per slot -- a 3-slot lookahead that covers the
        # score->exp->release round trip (~1.8us) at ~0.9us slots.
        LAG = 2
        recs = [None]
        deferred = []  # prev group's rec/oz closures, one per slot
        for gi, (m, h2, sb) in enumerate(combos):
            h = 2 * m + h2
            dbase = h2 * D
            ssl = slice(sb * 512, (sb + 1) * 512)
            acc = ps.tile([P, 512], F32, tag="acc", bufs=2, name=f"acc{gi}")
            exs = [None] * NP
            nfin = 0
            for tp in range(NP + LAG):
                # prev group's rec/oz, one per slot so they interleave
                # between this group's DVE exps instead of damming them
                if deferred and tp >= 1:
                    deferred.pop(0)()
                if tp < NP:
                    sc2 = ps.tile([P, 2, 512], F32, tag="big", bufs=3,
                                  name=f"sc{gi}_{tp}")
                    for tt in range(2):
                        t = 2 * tp + tt
                        nc.tensor.matmul(
                            sc2[:, tt, :],
                            lhsT=zqkT[t // 4][dbase:dbase + D, 2 + m,
                                              t % 4, :],
                            rhs=zqkT[sb][dbase:dbase + D, m, :, :]
                                .rearrange("p a b -> p (a b)"),
                            start=True, stop=True,
                            tile_position=(dbase, 0),
                        )
                    ex = exq.tile([P, 2, 512], F8, tag="ex",
                                  name=f"ex{gi}_{tp}")
                    sc_flat = sc2[:].rearrange("p a b -> p (a b)")
                    if (ex_idx % EXP_MOD) in EXP_DVE:
                        nc.vector.tensor_scalar(
                            out=ex[:].rearrange(
                                "p a b -> p (a b)").bitcast(I8),
                            in0=sc_flat,
                            scalar1=EXP_A, scalar2=EXP_B,
                            op0=ALU.mult, op1=ALU.add,
                        )
                    else:
                        nc.scalar.activation(
                            ex[:].rearrange("p a b -> p (a b)"),
                            sc_flat, AF.Exp, scale=INV_SQRT_D)
                    if _DEBUG and gi == 0 and tp == 0:
                        nc.sync.dma_start(out=dbg["dbg_ex0"][:], in_=ex[:])
                    ex_idx += 1
                    exs[tp] = ex
                if tp >= LAG:
                    tpp = tp - LAG
                    nc.tensor.matmul(
                        acc[:],
                        lhsT=vz[tpp][:, :, h, :, :].rearrange(
                            "p a b c -> p a (b c)"),
                        rhs=exs[tpp][:],
                        start=(tpp == 0), stop=(tpp == NP - 1),
                        perf_mode=DR,
                    )
                # one final-projection pair per slot once chunks are queued
                elif pending and nfin < 1 and tp >= 1:
                    emit_final_pair("big")
                    nfin += 1

            def _rec(gi=gi, acc=acc):
                rec = nrm.tile([D, 512], F32, tag="rec", bufs=2,
                               name=f"rec{gi}")
                nc.vector.reciprocal_approx_fast(rec[:], acc[0:D, :])
                recs[0] = rec

            def _oz(m=m, h2=h2, sb=sb, dbase=dbase, ssl=ssl, acc=acc):
                nc.vector.tensor_tensor(
                    out=ozb[dbase:dbase + D, m, ssl],
                    in0=acc[D:2 * D, :], in1=recs[0][:],
                    op=ALU.mult,
                )
                if m == 1 and h2 == 1:
                    pending.extend((mo, sb) for mo in range(0, E // P, 2))

            deferred.extend([_rec, _oz])

        while deferred:
            deferred.pop(0)()
        # drain: remaining projection pairs reuse the freed big psum tag;
        # copies strictly alternate engines so they run two-wide
        di = 0
        while pending:
            emit_final_pair("big", engine="a" if di % 2 == 0 else "d")
            di += 1


def _build_body(tc, xT8, w8, wc16d, yT, dbg=None):
    nc = tc.nc

    with (
        tc.tile_pool(name="const", bufs=1) as const,
        tc.tile_pool(name="wc", bufs=1) as wcp,
        tc.tile_pool(name="vz", bufs=1) as vzp,
        tc.tile_pool(name="zT", bufs=1) as zTp,
        tc.tile_pool(name="zs", bufs=1) as zsp,
        tc.tile_pool(name="cdz", bufs=1) as cdzp,
        tc.tile_pool(name="ps", bufs=1, space="PSUM") as ps,
    ):
        hp = const.tile([P, 1], F32)
        nc.vector.memset(hp[:], HALF_PI)
        warm = const.tile([P, 512], F32)
        nc.vector.memset(warm[:], 0.0)
        wc16 = wcp.tile([P, 2, E], F16)

        # vz pair tiles: [t-in-pair, head, (ones|z), 64]
        vz = [vzp.tile([P, 2, HG, 2, D], F8, tag=f"vz{j}", name=f"vz{j}")
              for j in range(NT // 2)]

        # zqkT per-sb buffers: [chunk(zq m0, zq m1, zk m0, zk m1), t', s]
        zqkT = [zTp.tile([P, 4, 4, P], F16, tag=f"zqkT{sb}", name=f"zqkT{sb}")
                for sb in range(4)]

        zqs = zsp.tile([P, NT, 3 * EG], F16, tag="zqs", name="zqs")[:]

        c_bufs = [cdzp.tile([P, 3 * EG], F16, tag=f"c{i}", name=f"c{i}")
                  for i in range(3)]
        dz_bufs = [cdzp.tile([P, 3 * EG], F16, tag=f"dz{i}", name=f"dz{i}")
                   for i in range(3)]
        for i in range(3):
            # c segment-boundary cols must stay 0 (reset trick); dz must be
            # 0 outside the boundary cols. These gate the first scans, so
            # they precede the vz ones-memsets on Pool.
            nc.gpsimd.memset(
                c_bufs[i][:].rearrange("p (s c) -> p s c", c=D)[:, :, 0:1],
                0.0)
            nc.gpsimd.memset(dz_bufs[i][:], 0.0)
        for j in range(NT // 2):
            nc.gpsimd.memset(vz[j][:, :, :, 0, :], 1.0)

        # ---------------- input DMA + phase A ----------------
        with (
            tc.tile_pool(name="x", bufs=1) as xp,
            tc.tile_pool(name="w", bufs=1) as wp,
        ):
            x8 = xp.tile([P, KC, S], F8, tag="x8", name="x8")
            w8s = wp.tile([P, KC, 3 * EG], F8, tag="w8", name="w8s")
            xTr = xT8.rearrange("(k p) s -> p k s", p=P)
            wTr = w8.rearrange("(k p) n -> p k n", p=P)
            # few big DMAs: HWDGE slot cost (625 ns) dominates small ones.
            # x lands s-chunk-major (all k per 512 s-cols) so tile t's
            # theta completes right after chunk t//4 -- the first Sin can
            # start ~4 us in instead of waiting the whole x transfer.
            nc.sync.dma_start(out=w8s[:], in_=wTr[:])
            for sb4 in range(4):
                ssl4 = slice(sb4 * 512, (sb4 + 1) * 512)
                nc.sync.dma_start(out=x8[:, :, ssl4], in_=xTr[:, :, ssl4])
            nc.sync.dma_start(
                out=wc16[:],
                in_=wc16d.rearrange("(m p) e -> p m e", p=P))

            _phase_a(tc, x8[:], w8s[:], hp, warm, c_bufs, dz_bufs, zqs, zqkT,
                     vz, ps)

            if _DEBUG:
                nc.sync.dma_start(out=dbg["dbg_c0"][:], in_=c_bufs[0][:])
                nc.sync.dma_start(out=dbg["dbg_zqs0"][:], in_=zqs[:, 0, :])
                nc.sync.dma_start(
                    out=dbg["dbg_zqkT0"][:],
                    in_=zqkT[0][:].rearrange("p a b c -> p (a b c)"))
                nc.sync.dma_start(
                    out=dbg["dbg_vz0"][:],
                    in_=vz[0][:].rearrange("p a h s d -> p (a h s d)"))

            # ---------------- phase B ----------------
            with tc.tile_pool(name="oz", bufs=1) as ozp:
                ozb = ozp.tile([P, 2, S], F16, tag="ozb", name="ozb")[:]
                _attention(tc, ozb, zqkT, vz, wc16[:], yT, ps, dbg)
                if _DEBUG:
                    nc.sync.dma_start(
                        out=dbg["dbg_oz0"][:],
                        in_=ozb[:, 0, :])


def build_bass():
    nc = bacc.Bacc(None, target_bir_lowering=False)
    xT8 = nc.dram_tensor("xT8", [E, S], F8, kind="ExternalInput")
    w8 = nc.dram_tensor("w8", [E, 3 * EG], F8, kind="ExternalInput")
    wc16d = nc.dram_tensor("wc16d", [EG, E], F16, kind="ExternalInput")
    yT = nc.dram_tensor("yT", [E, S], F16, kind="ExternalOutput")
    dbg = {}
    if _DEBUG:
        for nm, shp, dt in (("dbg_c0", [P, 3 * EG], F16),
                            ("dbg_zqs0", [P, 3 * EG], F16),
                            ("dbg_zqkT0", [P, 4 * 4 * P], F16),
                            ("dbg_vz0", [P, 2 * HG * 2 * D], F8),
                            ("dbg_ex0", [P, 2, 512], F8),
                            ("dbg_oz0", [P, S], F8)):
            dbg[nm] = nc.dram_tensor(nm, shp, dt, kind="ExternalOutput")[:]
    with tile.TileContext(nc) as tc:
        _build_body(tc, xT8[:], w8[:], wc16d[:], yT[:], dbg)
    nc.finalize()
    return nc


_NC_CACHE = None


def _get_nc():
    global _NC_CACHE
    if _NC_CACHE is None:
        _NC_CACHE = build_bass()
    return _NC_CACHE


def kernel(x, Wq, Wk, Wv, Wc, bc, **kw):
    import ml_dtypes
    f8 = ml_dtypes.float8_e4m3
    x = np.asarray(x, np.float32)
    in_maps = []
    for c in range(NCORES):
        b, g = divmod(c, NCORES // B)
        sl = slice(g * EG, (g + 1) * EG)
        wqkv = np.concatenate(
            [np.asarray(Wq)[sl, :].T, np.asarray(Wk)[sl, :].T,
             np.asarray(Wv)[sl, :].T], axis=1) * WSCALE
        in_maps.append({
            "xT8": np.ascontiguousarray(x[b].T).astype(f8),
            "w8": np.ascontiguousarray(wqkv).astype(f8),
            "wc16d": np.ascontiguousarray(
                np.asarray(Wc)[:, sl].T).astype(np.float16),
        })
    nc = _get_nc()
    res = run_bass_kernel_spmd(
        nc, in_maps, core_ids=list(range(NCORES)),
        trace=bool(int(os.environ.get("QK_TRACE", "0"))),
    )
    y = np.zeros((B, S, E), np.float32)
    for c in range(NCORES):
        b = c // (NCORES // B)
        y[b] += res.results[c]["yT"].astype(np.float32).T
    y += np.asarray(bc, np.float32)
    globals()["_LAST_RESULT"] = res
    return y
